# revision 47
# baseline (speedup 1.0000x reference)
"""Multi-head dot-product attention (B=2, Q=K=2048, EMB=2048, H=32, D=64) on 8 TRN2 cores.

Sharding: data parallel over batch (2) x tensor parallel over heads (4 groups of 8).
Core c handles batch c//4, heads 8*(c%4) .. 8*(c%4)+8. Each core computes a partial
output [2048, 2048] (its heads' contribution through wo); host sums the 4 head-group
partials per batch.

Single software-pipelined program per core (all matmuls bf16, T=2048 tokens,
HD=512 = 8 heads x 64):
  pre-phase: k^T/v projections (PE-dense; xkv streamed in quarter-chunks through
             8 concurrently-open psum accumulation groups) with exp(bias^T) on
             ScalarE underneath, plus q^T projection for the first q-chunk.
  main:      one flat pipeline over 256 iterations (4 q-chunks x 4 head-pairs x
             16 key-chunks). Per iteration: scores s^T[k,q] for the head pair
             (row-packed K=64 matmul pair), one merged exp on ScalarE, exp(bias)
             mul on DVE; the ctx^T accumulation (ones-augmented v, M=65) trails
             two iterations behind so ScalarE never waits on the PE queue.
             PE idle slots are filled with the remaining q-projection and the
             output-projection matmul groups, a few matmuls at a time.
  tail:      output projection for the last q-chunk.
Output partials are written bf16; the host sums them in fp32.
"""

import numpy as np
import ml_dtypes
from contextlib import ExitStack

import concourse.bass as bass
from concourse import bacc
import concourse.mybir as mybir
import concourse.tile as tile
from concourse.bass_utils import run_bass_kernel_spmd

BF16 = mybir.dt.bfloat16
F32 = mybir.dt.float32
AF = mybir.ActivationFunctionType

B, T, E = 2, 2048, 2048          # batch, tokens (Q=K), embed
H, D = 32, 64                     # total heads, head dim
NH = 8                            # heads per core
HD = NH * D                       # 512, per-core head-dim total
EC = E // 128                     # 16 contraction chunks
KC = T // 128                     # 16 key chunks
QCH = 512                         # attention q-chunk
NQC = T // QCH                    # 4 q-chunks
NPAIR = NH // 2                   # 4 head pairs
N_CORES = 8


def build_program():
    nc = bacc.Bacc("TRN2", target_bir_lowering=False, debug=False,
                   num_devices=N_CORES)

    xqT = nc.dram_tensor("xqT", [E, T], BF16, kind="ExternalInput").ap()
    xkvT = nc.dram_tensor("xkvT", [E, T], BF16, kind="ExternalInput").ap()
    biasT = nc.dram_tensor("biasT", [T, T], BF16, kind="ExternalInput").ap()
    wq = nc.dram_tensor("wq", [E, HD], BF16, kind="ExternalInput").ap()
    wk = nc.dram_tensor("wk", [E, HD], BF16, kind="ExternalInput").ap()
    wv = nc.dram_tensor("wv", [E, HD], BF16, kind="ExternalInput").ap()
    wo = nc.dram_tensor("wo", [HD, E], BF16, kind="ExternalInput").ap()
    out = nc.dram_tensor("out", [T, E], BF16, kind="ExternalOutput").ap()

    with tile.TileContext(nc) as tc, ExitStack() as ctx:
        persist = ctx.enter_context(tc.tile_pool(name="persist", bufs=1))
        kT_sb = persist.tile([128, NPAIR, T], BF16)       # k^T[d(2 heads), pair, t]
        v_sb = persist.tile([128, KC, NH, D + 1], BF16)   # v[k, kc, h, d] + ones col
        nc.vector.memset(v_sb[:, :, :, D:D + 1], 1.0)

        qT_pool = ctx.enter_context(tc.tile_pool(name="qtp", bufs=3))
        ctxT_pool = ctx.enter_context(tc.tile_pool(name="ctp", bufs=2))
        expb_pool = ctx.enter_context(tc.tile_pool(name="ebp", bufs=3))
        CTX_LAG = 2               # iterations between scores and ctx consumption
        rawb_pool = ctx.enter_context(tc.tile_pool(name="rbp", bufs=3))
        wq_pool = ctx.enter_context(tc.tile_pool(name="wqp", bufs=1))
        xq_pool = ctx.enter_context(tc.tile_pool(name="xqp", bufs=2))

        qT_t = [None] * NQC       # [128, NPAIR, QCH] bf16 per q-chunk
        ctxT_t = [None] * NQC     # [128, NPAIR, QCH] bf16 per q-chunk
        expb_t = [None] * NQC     # [128, KC, QCH] bf16 per q-chunk
        xq_t = [None] * NQC

        wq_sb = wq_pool.tile([128, EC, HD], BF16)

        def load_xq(qcc):
            t = xq_pool.tile([128, EC, QCH], BF16, name="xq_t", tag="xq")
            for half in range(2):
                nc.sync.dma_start(
                    out=t[:, half * 8:(half + 1) * 8, :],
                    in_=bass.AP(tensor=xqT.tensor,
                                offset=xqT.offset + qcc * QCH + half * 8 * 128 * T,
                                ap=[[T, 128], [128 * T, 8], [1, QCH]]))
            xq_t[qcc] = t

        def bias_exp_thunks(qcc):
            """Thunks: DMA raw bias columns for q-chunk qcc + exp into expb.
            The DMA for step k issues one thunk ahead of its exp so the exp
            never waits on its DMA inside ScalarE's in-order queue."""
            eb = expb_pool.tile([128, KC, QCH], BF16, name="expb_t", tag="expb")
            expb_t[qcc] = eb
            raws = [None] * 8

            def dma(kcg, qcc=qcc, raws=raws):
                raw = rawb_pool.tile([128, 2, QCH], BF16, name="rawb", tag="raw")
                nc.gpsimd.dma_start(
                    out=raw[:],
                    in_=bass.AP(tensor=biasT.tensor,
                                offset=biasT.offset + kcg * 256 * T + qcc * QCH,
                                ap=[[T, 128], [128 * T, 2], [1, QCH]]))
                raws[kcg] = raw

            thunks = []
            for kcg in range(8):
                def step(kcg=kcg, eb=eb, qcc=qcc, raws=raws):
                    if kcg == 0:
                        dma(0, qcc, raws)
                    if kcg < 7:
                        dma(kcg + 1, qcc, raws)
                    nc.scalar.activation(eb[:, kcg * 2:(kcg + 1) * 2, :],
                                         raws[kcg][:], AF.Exp)
                thunks.append(step)
            return thunks

        # ---------------- pre-phase: k/v projections ----------------
        wkv_pool = tc.alloc_tile_pool(name="wkv", bufs=1)
        xkv_pool = tc.alloc_tile_pool(name="xkv", bufs=4)
        pp = tc.alloc_tile_pool(name="pp", bufs=1, space="PSUM")

        wk_sb = wkv_pool.tile([128, EC, HD], BF16)
        wv_sb = wkv_pool.tile([128, EC, HD], BF16)
        # startup choreography: spread the first loads over all three DMA
        # paths in consumption order so the first kproj/vproj groups never wait
        nc.scalar.dma_start(
            out=wk_sb[:, 0:4, :],
            in_=wk[0:512, :].rearrange("(ec p) n -> p ec n", p=128))
        nc.scalar.dma_start(
            out=wv_sb[:, 0:4, :],
            in_=wv[0:512, :].rearrange("(ec p) n -> p ec n", p=128))
        for eg in (1, 2, 3):
            nc.gpsimd.dma_start(
                out=wk_sb[:, eg * 4:(eg + 1) * 4, :],
                in_=wk[eg * 512:(eg + 1) * 512, :].rearrange("(ec p) n -> p ec n", p=128))

        def load_xkv_quarter(tc4, ecq, eng=None):
            t = xkv_pool.tile([128, 4, HD], BF16, name="xkv_t", tag="xkv")
            (eng or nc.sync).dma_start(
                out=t[:],
                in_=bass.AP(tensor=xkvT.tensor,
                            offset=xkvT.offset + tc4 * 512 + ecq * 4 * 128 * T,
                            ap=[[T, 128], [128 * T, 4], [1, 512]]))
            return t

        pre_bias = (bias_exp_thunks(0) + bias_exp_thunks(1)
                    + bias_exp_thunks(2))
        # depth-2 xkv prefetch alternating over the two hw DMA queues
        pend_q = [load_xkv_quarter(0, 0, nc.sync),
                  load_xkv_quarter(0, 1, nc.scalar)]
        qidx = [2]
        for tc4 in range(4):
            if tc4 == 2:
                for eg in range(4):
                    nc.gpsimd.dma_start(
                        out=wq_sb[:, eg * 4:(eg + 1) * 4, :],
                        in_=wq[eg * 512:(eg + 1) * 512, :].rearrange(
                            "(ec p) n -> p ec n", p=128))
                load_xq(0)
            # 8 concurrently-open psum accumulation groups: kT (4 hdc) + v (4 sub)
            ps_k = [pp.tile([128, 512], F32, name=f"ppk{h}", tag=f"ppk{h}")
                    for h in range(NPAIR)]
            ps_v = [pp.tile([128, 512], F32, name=f"ppv{s}", tag=f"ppv{s}")
                    for s in range(4)]
            for ecq in range(4):
                cur_q = pend_q.pop(0)
                if tc4 == 0 and ecq > 0:
                    nc.scalar.dma_start(
                        out=wv_sb[:, ecq * 4:(ecq + 1) * 4, :],
                        in_=wv[ecq * 512:(ecq + 1) * 512, :].rearrange(
                            "(ec p) n -> p ec n", p=128))
                if qidx[0] < 16:
                    eng = (nc.sync, nc.scalar, nc.gpsimd)[qidx[0] % 3]
                    pend_q.append(load_xkv_quarter(qidx[0] // 4, qidx[0] % 4,
                                                   eng))
                    qidx[0] += 1
                if pre_bias and tc4 >= 1:
                    pre_bias.pop(0)()
                    if tc4 >= 2:
                        pre_bias.pop(0)()
                for hdc in range(NPAIR):
                    for e4 in range(4):
                        ec = ecq * 4 + e4
                        nc.tensor.matmul(ps_k[hdc][:],
                                         lhsT=wk_sb[:, ec, hdc * 128:(hdc + 1) * 128],
                                         rhs=cur_q[:, e4, :],
                                         start=(ec == 0), stop=(ec == EC - 1),
                                         skip_group_check=True)
                for sub in range(4):
                    for e4 in range(4):
                        ec = ecq * 4 + e4
                        nc.tensor.matmul(ps_v[sub][:],
                                         lhsT=cur_q[:, e4, sub * 128:(sub + 1) * 128],
                                         rhs=wv_sb[:, ec, :],
                                         start=(ec == 0), stop=(ec == EC - 1),
                                         skip_group_check=True)
            with tc.high_priority(offset=1_500_000):
                for hdc in range(NPAIR):
                    nc.vector.tensor_copy(
                        kT_sb[:, hdc, tc4 * 512:(tc4 + 1) * 512], ps_k[hdc][:])
                for sub in range(4):
                    nc.vector.tensor_copy(
                        v_sb[:, tc4 * 4 + sub, :, 0:D],
                        ps_v[sub].rearrange("p (h d) -> p h d", h=NH))

        # q-projection for q-chunk 0 (uses the pre-phase psum pool)
        qT_t[0] = qT_pool.tile([128, NPAIR, QCH], BF16, name="qT_t", tag="qT")
        for hdc in range(NPAIR):
            ps = pp.tile([128, 512], F32, name="pp_t", tag=f"ppk{hdc}")
            for ec in range(EC):
                nc.tensor.matmul(ps[:],
                                 lhsT=wq_sb[:, ec, hdc * 128:(hdc + 1) * 128],
                                 rhs=xq_t[0][:, ec, :],
                                 start=(ec == 0), stop=(ec == EC - 1))
            nc.vector.tensor_copy(qT_t[0][:, hdc, :], ps[:])
        for th in pre_bias:
            th()
        load_xq(1)

        pp.release()
        xkv_pool.release()
        wkv_pool.release()

        # ---------------- main attention phase ----------------
        wo_pool = ctx.enter_context(tc.tile_pool(name="wop", bufs=1))
        cpool = tc.alloc_tile_pool(name="cpsum", bufs=3, space="PSUM")
        fpool = tc.alloc_tile_pool(name="fpsum", bufs=1, space="PSUM")
        spool = tc.alloc_tile_pool(name="spsum", bufs=2, space="PSUM")
        atpool = ctx.enter_context(tc.tile_pool(name="atp", bufs=3))
        a2pool = ctx.enter_context(tc.tile_pool(name="a2p", bufs=4))
        ostage_pool = ctx.enter_context(tc.tile_pool(name="osp", bufs=2))
        norm_pool = ctx.enter_context(tc.tile_pool(name="nrm", bufs=1))

        wo_sb = wo_pool.tile([128, NPAIR, E], BF16)

        # ---- fill-work machinery: thunks emitted between attention matmuls ----
        def qproj_thunks(qcc):
            qT_t[qcc] = qT_pool.tile([128, NPAIR, QCH], BF16, name="qT_t", tag="qT")
            thunks = []
            for hdc in range(NPAIR):
                ps_box = [None]
                for ec in range(EC):
                    def mm(hdc=hdc, ec=ec, ps_box=ps_box, qcc=qcc):
                        if ec == 0:
                            ps_box[0] = fpool.tile([128, 512], F32, name="fp_t",
                                                   tag="fill")
                        nc.tensor.matmul(ps_box[0][:],
                                         lhsT=wq_sb[:, ec, hdc * 128:(hdc + 1) * 128],
                                         rhs=xq_t[qcc][:, ec, :],
                                         start=(ec == 0), stop=(ec == EC - 1),
                                         skip_group_check=True)
                    thunks.append(mm)

                def fin(hdc=hdc, ps_box=ps_box, qcc=qcc):
                    nc.scalar.copy(qT_t[qcc][:, hdc, :], ps_box[0][:])
                fin.is_fin = True
                thunks.append(fin)
            return thunks

        def outproj_thunks(qcc):
            """Output projection for q-chunk qcc (reads ctxT_t[qcc])."""
            thunks = []
            for t16 in range(QCH // 128):
                for ncol in range(E // 512):
                    ps_box = [None]
                    for pair in range(NPAIR):
                        def mm(pair=pair, t16=t16, ncol=ncol, ps_box=ps_box, qcc=qcc):
                            if pair == 0:
                                ps_box[0] = fpool.tile([128, 512], F32, name="fp_t",
                                                       tag="fill")
                            nc.tensor.matmul(
                                ps_box[0][:],
                                lhsT=ctxT_t[qcc][:, pair, t16 * 128:(t16 + 1) * 128],
                                rhs=wo_sb[:, pair, ncol * 512:(ncol + 1) * 512],
                                start=(pair == 0), stop=(pair == NPAIR - 1),
                                skip_group_check=True)
                        thunks.append(mm)

                    def fin(t16=t16, ncol=ncol, ps_box=ps_box, qcc=qcc):
                        ot = ostage_pool.tile([128, 512], BF16, name="ot_t", tag="ot")
                        nc.vector.tensor_copy(ot[:], ps_box[0][:])
                        nc.sync.dma_start(
                            out=out[(qcc * 4 + t16) * 128:(qcc * 4 + t16 + 1) * 128,
                                    ncol * 512:(ncol + 1) * 512],
                            in_=ot[:])
                    fin.is_fin = True
                    thunks.append(fin)
            return thunks

        fill_q = []

        def drain_fill(n):
            for _ in range(n):
                if not fill_q:
                    return
                fill_q.pop(0)()

        # ---- flat software pipeline over all (qcc, pair, kc) iterations ----
        flat = [(qcc, pair, kc)
                for qcc in range(NQC)
                for pair in ([1, 2, 3, 0] if qcc == NQC - 1 else range(NPAIR))
                for kc in range(KC)]
        NIT = len(flat)
        a2_ring = [None] * NIT   # a2 tile per iteration, consumed CTX_LAG iters later
        ctx_cur = [None]         # current AU's ctx psum pair

        def emit_scores(i):
            qcc, pair, kc = flat[i]
            with tc.high_priority(offset=1_000_000):
                _emit_scores_prio(i)

        def _emit_scores_prio(i):
            qcc, pair, kc = flat[i]
            s = spool.tile([128, 2 * QCH], F32, name="s_t", tag="s")
            for hh in range(2):
                pr = slice(hh * D, (hh + 1) * D)
                nc.tensor.matmul(
                    s[:, hh * QCH:(hh + 1) * QCH],
                    lhsT=kT_sb[pr, pair, kc * 128:(kc + 1) * 128],
                    rhs=qT_t[qcc][pr, pair, :],
                    start=True, stop=True)
            at = atpool.tile([128, 2 * QCH], BF16, name="at_t", tag="at")
            nc.scalar.activation(at[:], s[:], AF.Exp)
            a2 = a2pool.tile([128, 2 * QCH], BF16, name="a2_t", tag="a2")
            # one merged mul: expb block broadcast (stride-0) over both heads
            ebb = expb_t[qcc][:, kc, :].unsqueeze(1).broadcast_to([128, 2, QCH])
            nc.vector.tensor_mul(
                a2[:].rearrange("p (h q) -> p h q", h=2),
                at[:].rearrange("p (h q) -> p h q", h=2),
                ebb)
            a2_ring[i] = a2

        def emit_ctx(i):
            qcc, pair, kc = flat[i]
            if kc == 0:
                ctx_cur[0] = [cpool.tile([D + 1, QCH], F32, name=f"ctx{hh}",
                                         tag="ctx")
                              for hh in range(2)]
            a2 = a2_ring[i]
            a2_ring[i] = None
            for hh in range(2):
                nc.tensor.matmul(
                    ctx_cur[0][hh][0:D + 1, :],
                    lhsT=v_sb[:, kc, pair * 2 + hh, :],
                    rhs=a2[:, hh * QCH:(hh + 1) * QCH],
                    start=(kc == 0), stop=(kc == KC - 1))
            if kc == KC - 1:
                emit_normalize(qcc, pair, ctx_cur[0])

        def emit_normalize(qcc, pair, ctx_t):
            if (qcc, pair) == (NQC - 1, 0):
                # last AU: its normalize gates the tail -- jump every queue
                with tc.high_priority(offset=2_000_000):
                    _emit_normalize(qcc, pair, ctx_t)
            else:
                _emit_normalize(qcc, pair, ctx_t)

        def _emit_normalize(qcc, pair, ctx_t):
            # normalize ctx rows 0..63 by 1/ctx[64] (the attn row-sums),
            # reading the psum accumulators directly. DVE copies can shift
            # partition base, so the row-sum moves to partition 0 with a tiny
            # copy and the hh1 product writes straight to ctxT rows 64:128.
            # Both recip/broadcast chains are issued before either scale-mul
            # so the second chain's DVE work hides the first broadcast.
            recbs = []
            for hh in range(2):
                s1 = norm_pool.tile([1, QCH], F32, name="sum_t", tag=f"sum{hh}")
                nc.vector.tensor_copy(s1[:], ctx_t[hh][D:D + 1, :])
                rec = norm_pool.tile([1, QCH], F32, name="rec_t", tag=f"rec{hh}")
                nc.vector.reciprocal_approx_fast(out=rec[:], in_=s1[:])
                recb = norm_pool.tile([D, QCH], F32, name="recb_t",
                                      tag=f"recb{hh}")
                nc.gpsimd.partition_broadcast(recb[:], rec[:], channels=D)
                recbs.append(recb)
            for hh in range(2):
                nc.vector.tensor_mul(
                    ctxT_t[qcc][hh * D:(hh + 1) * D, pair, :],
                    ctx_t[hh][0:D, :], recbs[hh][:])

        bias_q = []   # pending bias-exp thunks, drained one per iteration

        prev_qcc = [-1]
        for i in range(NIT + CTX_LAG):
            if i < NIT:
                qcc, pair, kc = flat[i]
                if qcc != prev_qcc[0]:
                    prev_qcc[0] = qcc
                    # new q-chunk: allocate ctxT, queue fill + bias work
                    ctxT_t[qcc] = ctxT_pool.tile([128, NPAIR, QCH], BF16,
                                                 name="ctxT_t", tag="ctxT")
                    if qcc == 0:
                        fill_q.extend(qproj_thunks(1))
                        fill_q.extend(qproj_thunks(2))
                    elif qcc == 1:
                        fill_q.extend(qproj_thunks(3))
                        fill_q.extend(outproj_thunks(0))
                    else:
                        fill_q.extend(outproj_thunks(qcc - 1))
                    if qcc + 2 < NQC:
                        load_xq(qcc + 2)
                if kc == 0 and pair == 2 and qcc == 1:
                    bias_q.extend(bias_exp_thunks(3))
                if qcc == 0 and kc == 0 and pair in (0, 1):
                    nc.gpsimd.dma_start(
                        out=wo_sb[:, 2 * pair:2 * pair + 2, :],
                        in_=wo[pair * 256:(pair + 1) * 256, :].rearrange(
                            "(c p) n -> p c n", p=128))
                emit_scores(i)
                if bias_q and kc % 2 == 1:
                    bias_q.pop(0)()
            if i >= CTX_LAG:
                emit_ctx(i - CTX_LAG)
            # fill rate: drain evenly over the remaining iterations of this
            # q-chunk (+2 lag slack), at least 1, at most 4 per iteration
            rem_it = 16 * NPAIR - (flat[min(i, NIT - 1)][1] * KC +
                                   flat[min(i, NIT - 1)][2])
            rate = max(1, min(4, -(-len(fill_q) // max(1, rem_it))))
            drain_fill(rate)

        drain_fill(10 ** 9)
        spool.release()
        fpool.release()

        # ---------------- tail: output projection for the last q-chunk ----------------
        # tail psum opens in the released scores+fill banks: gated only by the
        # last exp's read, so pair-1/2/3 matmuls overlap the final normalize
        with tc.tile_pool(name="tailp", bufs=5, space="PSUM") as tailp:
            for gi in range(16):
                t16, ncol = gi // 4, gi % 4
                po = tailp.tile([128, 512], F32, name="po_t", tag="po")
                for j, pair in enumerate([1, 2, 3, 0]):
                    nc.tensor.matmul(
                        po[:],
                        lhsT=ctxT_t[NQC - 1][:, pair, t16 * 128:(t16 + 1) * 128],
                        rhs=wo_sb[:, pair, ncol * 512:(ncol + 1) * 512],
                        start=(j == 0), stop=(j == NPAIR - 1),
                        skip_group_check=True)
                ot = ostage_pool.tile([128, 512], BF16, name="ot_t", tag="ot")
                if gi % 2 == 0:
                    nc.vector.tensor_copy(ot[:], po[:])
                else:
                    nc.scalar.copy(ot[:], po[:])
                (nc.scalar if gi % 2 == 0 else nc.sync).dma_start(
                    out=out[((NQC - 1) * 4 + t16) * 128:
                            ((NQC - 1) * 4 + t16 + 1) * 128,
                            ncol * 512:(ncol + 1) * 512],
                    in_=ot[:])

        cpool.release()

    nc.compile()
    return nc


_NC_CACHE = {}


def kernel(inputs_q, inputs_kv, bias, wq, wk, wv, wo):
    bf16 = ml_dtypes.bfloat16
    inputs_q = np.asarray(inputs_q)
    inputs_kv = np.asarray(inputs_kv)
    bias = np.asarray(bias)
    # fold the reference's 1/sqrt(D) query scaling into wq
    wq_s = (np.asarray(wq).reshape(E, H * D) / np.sqrt(D)).astype(bf16)
    wk_s = np.asarray(wk).reshape(E, H * D).astype(bf16)
    wv_s = np.asarray(wv).reshape(E, H * D).astype(bf16)
    wo_s = np.asarray(wo).reshape(H * D, E).astype(bf16)

    # host-side layout marshaling: the kernel wants embed-major activations
    # and key-major bias (pure transposes, no math)
    xq_b = [np.ascontiguousarray(inputs_q[b].T).astype(bf16) for b in range(B)]
    xkv_b = [np.ascontiguousarray(inputs_kv[b].T).astype(bf16) for b in range(B)]
    bias_b = [np.ascontiguousarray(bias[b, 0].T).astype(bf16) for b in range(B)]

    in_maps = []
    for c in range(N_CORES):
        b, hg = c // 4, c % 4
        hs = slice(hg * HD, (hg + 1) * HD)
        in_maps.append({
            "xqT": xq_b[b],
            "xkvT": xkv_b[b],
            "biasT": bias_b[b],
            "wq": np.ascontiguousarray(wq_s[:, hs]),
            "wk": np.ascontiguousarray(wk_s[:, hs]),
            "wv": np.ascontiguousarray(wv_s[:, hs]),
            "wo": np.ascontiguousarray(wo_s[hs, :]),
        })

    if "nc" not in _NC_CACHE:
        _NC_CACHE["nc"] = build_program()
    nc = _NC_CACHE["nc"]

    res = run_bass_kernel_spmd(nc, in_maps, list(range(N_CORES)))
    outs = [np.asarray(r["out"], dtype=np.float32) for r in res.results]
    full = np.empty((B, T, E), dtype=np.float32)
    for b in range(B):
        full[b] = outs[4 * b] + outs[4 * b + 1] + outs[4 * b + 2] + outs[4 * b + 3]
    return full



# revision 48
# speedup vs baseline: 1.1542x; 1.1542x over previous
"""Multi-head dot-product attention (B=2, Q=K=2048, EMB=2048, H=32, D=64) on 8 TRN2 cores.

Sharding: data parallel over batch (2) x tensor parallel over heads (4 groups of 8).
Core c handles batch c//4, heads 8*(c%4) .. 8*(c%4)+8. Each core computes a partial
output [2048, 2048] (its heads' contribution through wo); host sums the 4 head-group
partials per batch.

Single software-pipelined program per core (all matmuls bf16, T=2048 tokens,
HD=512 = 8 heads x 64):
  pre-phase: k^T/v projections (PE-dense; xkv streamed in quarter-chunks through
             8 concurrently-open psum accumulation groups) with exp(bias^T) on
             ScalarE underneath, plus q^T projection for the first q-chunk.
  main:      one flat pipeline over 256 iterations (4 q-chunks x 4 head-pairs x
             16 key-chunks). Per iteration: scores s^T[k,q] for the head pair
             (row-packed K=64 matmul pair), one merged exp on ScalarE, exp(bias)
             mul on DVE; the ctx^T accumulation (ones-augmented v, M=65) trails
             two iterations behind so ScalarE never waits on the PE queue.
             PE idle slots are filled with the remaining q-projection and the
             output-projection matmul groups, a few matmuls at a time.
  tail:      output projection for the last q-chunk.
Output partials are written bf16; the host sums them in fp32.
"""

import numpy as np
import ml_dtypes
from contextlib import ExitStack

import concourse.bass as bass
from concourse import bacc
import concourse.mybir as mybir
import concourse.tile as tile
from concourse.bass_utils import run_bass_kernel_spmd

BF16 = mybir.dt.bfloat16
F32 = mybir.dt.float32
AF = mybir.ActivationFunctionType

B, T, E = 2, 2048, 2048          # batch, tokens (Q=K), embed
H, D = 32, 64                     # total heads, head dim
NH = 8                            # heads per core
HD = NH * D                       # 512, per-core head-dim total
EC = E // 128                     # 16 contraction chunks
KC = T // 128                     # 16 key chunks
QCH = 512                         # attention q-chunk
NQC = T // QCH                    # 4 q-chunks
NPAIR = NH // 2                   # 4 head pairs
N_CORES = 8


def build_program():
    nc = bacc.Bacc("TRN2", target_bir_lowering=False, debug=False,
                   num_devices=N_CORES)

    xqT = nc.dram_tensor("xqT", [E, T], BF16, kind="ExternalInput").ap()
    xkvT = nc.dram_tensor("xkvT", [E, T], BF16, kind="ExternalInput").ap()
    biasT = nc.dram_tensor("biasT", [T, T], BF16, kind="ExternalInput").ap()
    wq = nc.dram_tensor("wq", [E, HD], BF16, kind="ExternalInput").ap()
    wk = nc.dram_tensor("wk", [E, HD], BF16, kind="ExternalInput").ap()
    wv = nc.dram_tensor("wv", [E, HD], BF16, kind="ExternalInput").ap()
    wo = nc.dram_tensor("wo", [HD, E], BF16, kind="ExternalInput").ap()
    out = nc.dram_tensor("out", [T, E], BF16, kind="ExternalOutput").ap()

    with tile.TileContext(nc) as tc, ExitStack() as ctx:
        persist = ctx.enter_context(tc.tile_pool(name="persist", bufs=1))
        kT_sb = persist.tile([128, NPAIR, T], BF16)       # k^T[d(2 heads), pair, t]
        v_sb = persist.tile([128, KC, NH, D + 1], BF16)   # v[k, kc, h, d] + ones col
        nc.vector.memset(v_sb[:, :, :, D:D + 1], 1.0)

        qT_pool = ctx.enter_context(tc.tile_pool(name="qtp", bufs=3))
        ctxT_pool = ctx.enter_context(tc.tile_pool(name="ctp", bufs=2))
        expb_pool = ctx.enter_context(tc.tile_pool(name="ebp", bufs=3))
        CTX_LAG = 2               # iterations between scores and ctx consumption
        rawb_pool = ctx.enter_context(tc.tile_pool(name="rbp", bufs=3))
        wq_pool = ctx.enter_context(tc.tile_pool(name="wqp", bufs=1))
        xq_pool = ctx.enter_context(tc.tile_pool(name="xqp", bufs=2))

        qT_t = [None] * NQC       # [128, NPAIR, QCH] bf16 per q-chunk
        ctxT_t = [None] * NQC     # [128, NPAIR, QCH] bf16 per q-chunk
        expb_t = [None] * NQC     # [128, KC, QCH] bf16 per q-chunk
        xq_t = [None] * NQC

        wq_sb = wq_pool.tile([128, EC, HD], BF16)

        def load_xq(qcc):
            t = xq_pool.tile([128, EC, QCH], BF16, name="xq_t", tag="xq")
            for half in range(2):
                nc.sync.dma_start(
                    out=t[:, half * 8:(half + 1) * 8, :],
                    in_=bass.AP(tensor=xqT.tensor,
                                offset=xqT.offset + qcc * QCH + half * 8 * 128 * T,
                                ap=[[T, 128], [128 * T, 8], [1, QCH]]))
            xq_t[qcc] = t

        def bias_exp_thunks(qcc):
            """Thunks: DMA raw bias columns for q-chunk qcc + exp into expb.
            The DMA for step k issues one thunk ahead of its exp so the exp
            never waits on its DMA inside ScalarE's in-order queue."""
            eb = expb_pool.tile([128, KC, QCH], BF16, name="expb_t", tag="expb")
            expb_t[qcc] = eb
            raws = [None] * 8

            def dma(kcg, qcc=qcc, raws=raws):
                raw = rawb_pool.tile([128, 2, QCH], BF16, name="rawb", tag="raw")
                nc.gpsimd.dma_start(
                    out=raw[:],
                    in_=bass.AP(tensor=biasT.tensor,
                                offset=biasT.offset + kcg * 256 * T + qcc * QCH,
                                ap=[[T, 128], [128 * T, 2], [1, QCH]]))
                raws[kcg] = raw

            thunks = []
            for kcg in range(8):
                def step(kcg=kcg, eb=eb, qcc=qcc, raws=raws):
                    if kcg == 0:
                        dma(0, qcc, raws)
                    if kcg < 7:
                        dma(kcg + 1, qcc, raws)
                    nc.scalar.activation(eb[:, kcg * 2:(kcg + 1) * 2, :],
                                         raws[kcg][:], AF.Exp)
                thunks.append(step)
            return thunks

        # ---------------- pre-phase: k/v projections ----------------
        wkv_pool = tc.alloc_tile_pool(name="wkv", bufs=1)
        xkv_pool = tc.alloc_tile_pool(name="xkv", bufs=4)
        pp = tc.alloc_tile_pool(name="pp", bufs=1, space="PSUM")

        wk_sb = wkv_pool.tile([128, EC, HD], BF16)
        wv_sb = wkv_pool.tile([128, EC, HD], BF16)
        # startup choreography: spread the first loads over all three DMA
        # paths in consumption order so the first kproj/vproj groups never wait
        nc.scalar.dma_start(
            out=wk_sb[:, 0:4, :],
            in_=wk[0:512, :].rearrange("(ec p) n -> p ec n", p=128))
        nc.scalar.dma_start(
            out=wv_sb[:, 0:4, :],
            in_=wv[0:512, :].rearrange("(ec p) n -> p ec n", p=128))
        for eg in (1, 2, 3):
            nc.gpsimd.dma_start(
                out=wk_sb[:, eg * 4:(eg + 1) * 4, :],
                in_=wk[eg * 512:(eg + 1) * 512, :].rearrange("(ec p) n -> p ec n", p=128))

        def load_xkv_quarter(tc4, ecq, eng=None):
            t = xkv_pool.tile([128, 4, HD], BF16, name="xkv_t", tag="xkv")
            (eng or nc.sync).dma_start(
                out=t[:],
                in_=bass.AP(tensor=xkvT.tensor,
                            offset=xkvT.offset + tc4 * 512 + ecq * 4 * 128 * T,
                            ap=[[T, 128], [128 * T, 4], [1, 512]]))
            return t

        pre_bias = (bias_exp_thunks(0) + bias_exp_thunks(1)
                    + bias_exp_thunks(2))
        # depth-2 xkv prefetch alternating over the two hw DMA queues
        pend_q = [load_xkv_quarter(0, 0, nc.sync),
                  load_xkv_quarter(0, 1, nc.scalar)]
        qidx = [2]
        for tc4 in range(4):
            if tc4 == 2:
                for eg in range(4):
                    nc.gpsimd.dma_start(
                        out=wq_sb[:, eg * 4:(eg + 1) * 4, :],
                        in_=wq[eg * 512:(eg + 1) * 512, :].rearrange(
                            "(ec p) n -> p ec n", p=128))
                load_xq(0)
            # 8 concurrently-open psum accumulation groups: kT (4 hdc) + v (4 sub)
            ps_k = [pp.tile([128, 512], F32, name=f"ppk{h}", tag=f"ppk{h}")
                    for h in range(NPAIR)]
            ps_v = [pp.tile([128, 512], F32, name=f"ppv{s}", tag=f"ppv{s}")
                    for s in range(4)]
            for ecq in range(4):
                cur_q = pend_q.pop(0)
                if tc4 == 0 and ecq > 0:
                    nc.scalar.dma_start(
                        out=wv_sb[:, ecq * 4:(ecq + 1) * 4, :],
                        in_=wv[ecq * 512:(ecq + 1) * 512, :].rearrange(
                            "(ec p) n -> p ec n", p=128))
                if qidx[0] < 16:
                    eng = (nc.sync, nc.scalar, nc.gpsimd)[qidx[0] % 3]
                    pend_q.append(load_xkv_quarter(qidx[0] // 4, qidx[0] % 4,
                                                   eng))
                    qidx[0] += 1
                if pre_bias and tc4 >= 1:
                    pre_bias.pop(0)()
                    if tc4 >= 2:
                        pre_bias.pop(0)()
                for hdc in range(NPAIR):
                    for e4 in range(4):
                        ec = ecq * 4 + e4
                        nc.tensor.matmul(ps_k[hdc][:],
                                         lhsT=wk_sb[:, ec, hdc * 128:(hdc + 1) * 128],
                                         rhs=cur_q[:, e4, :],
                                         start=(ec == 0), stop=(ec == EC - 1),
                                         skip_group_check=True)
                for sub in range(4):
                    for e4 in range(4):
                        ec = ecq * 4 + e4
                        nc.tensor.matmul(ps_v[sub][:],
                                         lhsT=cur_q[:, e4, sub * 128:(sub + 1) * 128],
                                         rhs=wv_sb[:, ec, :],
                                         start=(ec == 0), stop=(ec == EC - 1),
                                         skip_group_check=True)
            with tc.high_priority(offset=1_500_000):
                for hdc in range(NPAIR):
                    nc.vector.tensor_copy(
                        kT_sb[:, hdc, tc4 * 512:(tc4 + 1) * 512], ps_k[hdc][:])
                for sub in range(4):
                    nc.vector.tensor_copy(
                        v_sb[:, tc4 * 4 + sub, :, 0:D],
                        ps_v[sub].rearrange("p (h d) -> p h d", h=NH))

        # q-projection for q-chunk 0 (uses the pre-phase psum pool)
        qT_t[0] = qT_pool.tile([128, NPAIR, QCH], BF16, name="qT_t", tag="qT")
        for hdc in range(NPAIR):
            ps = pp.tile([128, 512], F32, name="pp_t", tag=f"ppk{hdc}")
            for ec in range(EC):
                nc.tensor.matmul(ps[:],
                                 lhsT=wq_sb[:, ec, hdc * 128:(hdc + 1) * 128],
                                 rhs=xq_t[0][:, ec, :],
                                 start=(ec == 0), stop=(ec == EC - 1))
            nc.vector.tensor_copy(qT_t[0][:, hdc, :], ps[:])
        for th in pre_bias:
            th()
        load_xq(1)

        pp.release()
        xkv_pool.release()
        wkv_pool.release()

        # ---------------- main attention phase ----------------
        wo_pool = ctx.enter_context(tc.tile_pool(name="wop", bufs=1))
        cpool = tc.alloc_tile_pool(name="cpsum", bufs=3, space="PSUM")
        fpool = tc.alloc_tile_pool(name="fpsum", bufs=1, space="PSUM")
        spool = tc.alloc_tile_pool(name="spsum", bufs=2, space="PSUM")
        atpool = ctx.enter_context(tc.tile_pool(name="atp", bufs=2))
        a2pool = ctx.enter_context(tc.tile_pool(name="a2p", bufs=3))
        ostage_pool = ctx.enter_context(tc.tile_pool(name="osp", bufs=2))
        norm_pool = ctx.enter_context(tc.tile_pool(name="nrm", bufs=1))

        wo_sb = wo_pool.tile([128, NPAIR, E], BF16)

        # ---- fill-work machinery: thunks emitted between attention matmuls ----
        def qproj_thunks(qcc):
            qT_t[qcc] = qT_pool.tile([128, NPAIR, QCH], BF16, name="qT_t", tag="qT")
            thunks = []
            for hdc in range(NPAIR):
                ps_box = [None]
                for ec in range(EC):
                    def mm(hdc=hdc, ec=ec, ps_box=ps_box, qcc=qcc):
                        if ec == 0:
                            ps_box[0] = fpool.tile([128, 512], F32, name="fp_t",
                                                   tag="fill")
                        nc.tensor.matmul(ps_box[0][:],
                                         lhsT=wq_sb[:, ec, hdc * 128:(hdc + 1) * 128],
                                         rhs=xq_t[qcc][:, ec, :],
                                         start=(ec == 0), stop=(ec == EC - 1),
                                         skip_group_check=True)
                    thunks.append(mm)

                def fin(hdc=hdc, ps_box=ps_box, qcc=qcc):
                    nc.scalar.copy(qT_t[qcc][:, hdc, :], ps_box[0][:])
                fin.is_fin = True
                thunks.append(fin)
            return thunks

        def outproj_thunks(qcc):
            """Output projection for q-chunk qcc (reads ctxT_t[qcc])."""
            thunks = []
            for t16 in range(QCH // 128):
                for ncol in range(E // 512):
                    ps_box = [None]
                    for pair in range(NPAIR):
                        def mm(pair=pair, t16=t16, ncol=ncol, ps_box=ps_box, qcc=qcc):
                            if pair == 0:
                                ps_box[0] = fpool.tile([128, 512], F32, name="fp_t",
                                                       tag="fill")
                            nc.tensor.matmul(
                                ps_box[0][:],
                                lhsT=ctxT_t[qcc][:, pair, t16 * 128:(t16 + 1) * 128],
                                rhs=wo_sb[:, pair, ncol * 512:(ncol + 1) * 512],
                                start=(pair == 0), stop=(pair == NPAIR - 1),
                                skip_group_check=True)
                        thunks.append(mm)

                    def fin(t16=t16, ncol=ncol, ps_box=ps_box, qcc=qcc):
                        ot = ostage_pool.tile([128, 512], BF16, name="ot_t", tag="ot")
                        nc.vector.tensor_copy(ot[:], ps_box[0][:])
                        nc.sync.dma_start(
                            out=out[(qcc * 4 + t16) * 128:(qcc * 4 + t16 + 1) * 128,
                                    ncol * 512:(ncol + 1) * 512],
                            in_=ot[:])
                    fin.is_fin = True
                    thunks.append(fin)
            return thunks

        fill_q = []

        def drain_fill(n):
            for _ in range(n):
                if not fill_q:
                    return
                fill_q.pop(0)()

        # ---- flat software pipeline over all (qcc, pair, kc) iterations ----
        flat = [(qcc, pair, kc)
                for qcc in range(NQC)
                for pair in ([1, 2, 3, 0] if qcc == NQC - 1 else range(NPAIR))
                for kc in range(KC)]
        NIT = len(flat)
        a2_ring = [None] * NIT   # a2 tile per iteration, consumed CTX_LAG iters later
        ctx_cur = [None]         # current AU's ctx psum pair

        def emit_scores(i):
            qcc, pair, kc = flat[i]
            with tc.high_priority(offset=1_000_000):
                _emit_scores_prio(i)

        def _emit_scores_prio(i):
            qcc, pair, kc = flat[i]
            s = spool.tile([128, 2 * QCH], F32, name="s_t", tag="s")
            for hh in range(2):
                pr = slice(hh * D, (hh + 1) * D)
                nc.tensor.matmul(
                    s[:, hh * QCH:(hh + 1) * QCH],
                    lhsT=kT_sb[pr, pair, kc * 128:(kc + 1) * 128],
                    rhs=qT_t[qcc][pr, pair, :],
                    start=True, stop=True)
            at = atpool.tile([128, 2 * QCH], BF16, name="at_t", tag="at")
            nc.scalar.activation(at[:], s[:], AF.Exp)
            a2 = a2pool.tile([128, 2 * QCH], BF16, name="a2_t", tag="a2")
            # one merged mul: expb block broadcast (stride-0) over both heads
            ebb = expb_t[qcc][:, kc, :].unsqueeze(1).broadcast_to([128, 2, QCH])
            nc.vector.tensor_mul(
                a2[:].rearrange("p (h q) -> p h q", h=2),
                at[:].rearrange("p (h q) -> p h q", h=2),
                ebb)
            a2_ring[i] = a2

        def emit_ctx(i):
            qcc, pair, kc = flat[i]
            if kc == 0:
                ctx_cur[0] = [cpool.tile([D + 1, QCH], F32, name=f"ctx{hh}",
                                         tag="ctx")
                              for hh in range(2)]
            a2 = a2_ring[i]
            a2_ring[i] = None
            for hh in range(2):
                nc.tensor.matmul(
                    ctx_cur[0][hh][0:D + 1, :],
                    lhsT=v_sb[:, kc, pair * 2 + hh, :],
                    rhs=a2[:, hh * QCH:(hh + 1) * QCH],
                    start=(kc == 0), stop=(kc == KC - 1))
            if kc == KC - 1:
                emit_normalize(qcc, pair, ctx_cur[0])

        def emit_normalize(qcc, pair, ctx_t):
            if (qcc, pair) == (NQC - 1, 0):
                # last AU: its normalize gates the tail -- jump every queue
                with tc.high_priority(offset=2_000_000):
                    _emit_normalize(qcc, pair, ctx_t)
            else:
                _emit_normalize(qcc, pair, ctx_t)

        def _emit_normalize(qcc, pair, ctx_t):
            # normalize ctx rows 0..63 by 1/ctx[64] (the attn row-sums),
            # reading the psum accumulators directly. DVE copies can shift
            # partition base, so the row-sum moves to partition 0 with a tiny
            # copy and the hh1 product writes straight to ctxT rows 64:128.
            # Both recip/broadcast chains are issued before either scale-mul
            # so the second chain's DVE work hides the first broadcast.
            recbs = []
            for hh in range(2):
                s1 = norm_pool.tile([1, QCH], F32, name="sum_t", tag=f"sum{hh}")
                nc.vector.tensor_copy(s1[:], ctx_t[hh][D:D + 1, :])
                rec = norm_pool.tile([1, QCH], F32, name="rec_t", tag=f"rec{hh}")
                nc.vector.reciprocal_approx_fast(out=rec[:], in_=s1[:])
                recb = norm_pool.tile([D, QCH], F32, name="recb_t",
                                      tag=f"recb{hh}")
                nc.gpsimd.partition_broadcast(recb[:], rec[:], channels=D)
                recbs.append(recb)
            for hh in range(2):
                nc.vector.tensor_mul(
                    ctxT_t[qcc][hh * D:(hh + 1) * D, pair, :],
                    ctx_t[hh][0:D, :], recbs[hh][:])

        bias_q = []   # pending bias-exp thunks, drained one per iteration

        prev_qcc = [-1]
        for i in range(NIT + CTX_LAG):
            if i < NIT:
                qcc, pair, kc = flat[i]
                if qcc != prev_qcc[0]:
                    prev_qcc[0] = qcc
                    # new q-chunk: allocate ctxT, queue fill + bias work
                    ctxT_t[qcc] = ctxT_pool.tile([128, NPAIR, QCH], BF16,
                                                 name="ctxT_t", tag="ctxT")
                    if qcc == 0:
                        fill_q.extend(qproj_thunks(1))
                        fill_q.extend(qproj_thunks(2))
                    elif qcc == 1:
                        fill_q.extend(qproj_thunks(3))
                        fill_q.extend(outproj_thunks(0))
                    else:
                        fill_q.extend(outproj_thunks(qcc - 1))
                    if qcc + 2 < NQC:
                        load_xq(qcc + 2)
                if kc == 0 and pair == 2 and qcc == 1:
                    bias_q.extend(bias_exp_thunks(3))
                if qcc == 0 and kc == 0 and pair in (0, 1):
                    nc.gpsimd.dma_start(
                        out=wo_sb[:, 2 * pair:2 * pair + 2, :],
                        in_=wo[pair * 256:(pair + 1) * 256, :].rearrange(
                            "(c p) n -> p c n", p=128))
                emit_scores(i)
                if bias_q and kc % 2 == 1:
                    bias_q.pop(0)()
            if i >= CTX_LAG:
                emit_ctx(i - CTX_LAG)
            # fill rate: drain evenly over the remaining iterations of this
            # q-chunk (+2 lag slack), at least 1, at most 4 per iteration
            rem_it = 16 * NPAIR - (flat[min(i, NIT - 1)][1] * KC +
                                   flat[min(i, NIT - 1)][2])
            rate = max(1, min(4, -(-len(fill_q) // max(1, rem_it))))
            drain_fill(rate)

        drain_fill(10 ** 9)
        spool.release()
        fpool.release()

        # ---------------- tail: output projection for the last q-chunk ----------------
        # tail psum opens in the released scores+fill banks: gated only by the
        # last exp's read, so pair-1/2/3 matmuls overlap the final normalize
        with tc.tile_pool(name="tailp", bufs=5, space="PSUM") as tailp:
            for gi in range(16):
                t16, ncol = gi // 4, gi % 4
                po = tailp.tile([128, 512], F32, name="po_t", tag="po")
                for j, pair in enumerate([1, 2, 3, 0]):
                    nc.tensor.matmul(
                        po[:],
                        lhsT=ctxT_t[NQC - 1][:, pair, t16 * 128:(t16 + 1) * 128],
                        rhs=wo_sb[:, pair, ncol * 512:(ncol + 1) * 512],
                        start=(j == 0), stop=(j == NPAIR - 1),
                        skip_group_check=True)
                ot = ostage_pool.tile([128, 512], BF16, name="ot_t", tag="ot")
                if gi % 2 == 0:
                    nc.vector.tensor_copy(ot[:], po[:])
                else:
                    nc.scalar.copy(ot[:], po[:])
                (nc.scalar if gi % 2 == 0 else nc.sync).dma_start(
                    out=out[((NQC - 1) * 4 + t16) * 128:
                            ((NQC - 1) * 4 + t16 + 1) * 128,
                            ncol * 512:(ncol + 1) * 512],
                    in_=ot[:])

        cpool.release()

    nc.compile()
    return nc


_NC_CACHE = {}


def kernel(inputs_q, inputs_kv, bias, wq, wk, wv, wo):
    bf16 = ml_dtypes.bfloat16
    inputs_q = np.asarray(inputs_q)
    inputs_kv = np.asarray(inputs_kv)
    bias = np.asarray(bias)
    # fold the reference's 1/sqrt(D) query scaling into wq
    wq_s = (np.asarray(wq).reshape(E, H * D) / np.sqrt(D)).astype(bf16)
    wk_s = np.asarray(wk).reshape(E, H * D).astype(bf16)
    wv_s = np.asarray(wv).reshape(E, H * D).astype(bf16)
    wo_s = np.asarray(wo).reshape(H * D, E).astype(bf16)

    # host-side layout marshaling: the kernel wants embed-major activations
    # and key-major bias (pure transposes, no math)
    xq_b = [np.ascontiguousarray(inputs_q[b].T).astype(bf16) for b in range(B)]
    xkv_b = [np.ascontiguousarray(inputs_kv[b].T).astype(bf16) for b in range(B)]
    bias_b = [np.ascontiguousarray(bias[b, 0].T).astype(bf16) for b in range(B)]

    in_maps = []
    for c in range(N_CORES):
        b, hg = c // 4, c % 4
        hs = slice(hg * HD, (hg + 1) * HD)
        in_maps.append({
            "xqT": xq_b[b],
            "xkvT": xkv_b[b],
            "biasT": bias_b[b],
            "wq": np.ascontiguousarray(wq_s[:, hs]),
            "wk": np.ascontiguousarray(wk_s[:, hs]),
            "wv": np.ascontiguousarray(wv_s[:, hs]),
            "wo": np.ascontiguousarray(wo_s[hs, :]),
        })

    if "nc" not in _NC_CACHE:
        _NC_CACHE["nc"] = build_program()
    nc = _NC_CACHE["nc"]

    res = run_bass_kernel_spmd(nc, in_maps, list(range(N_CORES)))
    outs = [np.asarray(r["out"], dtype=np.float32) for r in res.results]
    full = np.empty((B, T, E), dtype=np.float32)
    for b in range(B):
        full[b] = outs[4 * b] + outs[4 * b + 1] + outs[4 * b + 2] + outs[4 * b + 3]
    return full



# revision 49
# speedup vs baseline: 1.1802x; 1.0225x over previous
"""Multi-head dot-product attention (B=2, Q=K=2048, EMB=2048, H=32, D=64) on 8 TRN2 cores.

Sharding: data parallel over batch (2) x tensor parallel over heads (4 groups of 8).
Core c handles batch c//4, heads 8*(c%4) .. 8*(c%4)+8. Each core computes a partial
output [2048, 2048] (its heads' contribution through wo); host sums the 4 head-group
partials per batch.

Single software-pipelined program per core (all matmuls bf16, T=2048 tokens,
HD=512 = 8 heads x 64):
  pre-phase: k^T/v projections (PE-dense; xkv streamed in quarter-chunks through
             8 concurrently-open psum accumulation groups) with exp(bias^T) on
             ScalarE underneath, plus q^T projection for the first q-chunk.
  main:      one flat pipeline over 256 iterations (4 q-chunks x 4 head-pairs x
             16 key-chunks). Per iteration: scores s^T[k,q] for the head pair
             (row-packed K=64 matmul pair), one merged exp on ScalarE, exp(bias)
             mul on DVE; the ctx^T accumulation (ones-augmented v, M=65) trails
             two iterations behind so ScalarE never waits on the PE queue.
             PE idle slots are filled with the remaining q-projection and the
             output-projection matmul groups, a few matmuls at a time.
  tail:      output projection for the last q-chunk.
Output partials are written bf16; the host sums them in fp32.
"""

import numpy as np
import ml_dtypes
from contextlib import ExitStack

import concourse.bass as bass
from concourse import bacc
import concourse.mybir as mybir
import concourse.tile as tile
from concourse.bass_utils import run_bass_kernel_spmd

BF16 = mybir.dt.bfloat16
F32 = mybir.dt.float32
AF = mybir.ActivationFunctionType

B, T, E = 2, 2048, 2048          # batch, tokens (Q=K), embed
H, D = 32, 64                     # total heads, head dim
NH = 8                            # heads per core
HD = NH * D                       # 512, per-core head-dim total
EC = E // 128                     # 16 contraction chunks
KC = T // 128                     # 16 key chunks
QCH = 512                         # attention q-chunk
NQC = T // QCH                    # 4 q-chunks
NPAIR = NH // 2                   # 4 head pairs
N_CORES = 8


def build_program():
    nc = bacc.Bacc("TRN2", target_bir_lowering=False, debug=False,
                   num_devices=N_CORES)

    xqT = nc.dram_tensor("xqT", [E, T], BF16, kind="ExternalInput").ap()
    xkvT = nc.dram_tensor("xkvT", [E, T], BF16, kind="ExternalInput").ap()
    biasT = nc.dram_tensor("biasT", [T, T], BF16, kind="ExternalInput").ap()
    wq = nc.dram_tensor("wq", [E, HD], BF16, kind="ExternalInput").ap()
    wk = nc.dram_tensor("wk", [E, HD], BF16, kind="ExternalInput").ap()
    wv = nc.dram_tensor("wv", [E, HD], BF16, kind="ExternalInput").ap()
    wo = nc.dram_tensor("wo", [HD, E], BF16, kind="ExternalInput").ap()
    out = nc.dram_tensor("out", [T, E], BF16, kind="ExternalOutput").ap()

    with tile.TileContext(nc) as tc, ExitStack() as ctx:
        persist = ctx.enter_context(tc.tile_pool(name="persist", bufs=1))
        kT_sb = persist.tile([128, NPAIR, T], BF16)       # k^T[d(2 heads), pair, t]
        v_sb = persist.tile([128, KC, NH, D + 1], BF16)   # v[k, kc, h, d] + ones col
        nc.vector.memset(v_sb[:, :, :, D:D + 1], 1.0)

        qT_pool = ctx.enter_context(tc.tile_pool(name="qtp", bufs=3))
        ctxT_pool = ctx.enter_context(tc.tile_pool(name="ctp", bufs=2))
        expb_pool = ctx.enter_context(tc.tile_pool(name="ebp", bufs=3))
        CTX_LAG = 2               # iterations between scores and ctx consumption
        rawb_pool = ctx.enter_context(tc.tile_pool(name="rbp", bufs=3))
        wq_pool = ctx.enter_context(tc.tile_pool(name="wqp", bufs=1))
        xq_pool = ctx.enter_context(tc.tile_pool(name="xqp", bufs=2))

        qT_t = [None] * NQC       # [128, NPAIR, QCH] bf16 per q-chunk
        ctxT_t = [None] * NQC     # [128, NPAIR, QCH] bf16 per q-chunk
        expb_t = [None] * NQC     # [128, KC, QCH] bf16 per q-chunk
        xq_t = [None] * NQC

        wq_sb = wq_pool.tile([128, EC, HD], BF16)

        def load_xq(qcc):
            t = xq_pool.tile([128, EC, QCH], BF16, name="xq_t", tag="xq")
            for half in range(2):
                nc.sync.dma_start(
                    out=t[:, half * 8:(half + 1) * 8, :],
                    in_=bass.AP(tensor=xqT.tensor,
                                offset=xqT.offset + qcc * QCH + half * 8 * 128 * T,
                                ap=[[T, 128], [128 * T, 8], [1, QCH]]))
            xq_t[qcc] = t

        def bias_exp_thunks(qcc):
            """Thunks: DMA raw bias columns for q-chunk qcc + exp into expb.
            The DMA for step k issues one thunk ahead of its exp so the exp
            never waits on its DMA inside ScalarE's in-order queue."""
            eb = expb_pool.tile([128, KC, QCH], BF16, name="expb_t", tag="expb")
            expb_t[qcc] = eb
            raws = [None] * 8

            def dma(kcg, qcc=qcc, raws=raws):
                raw = rawb_pool.tile([128, 2, QCH], BF16, name="rawb", tag="raw")
                nc.gpsimd.dma_start(
                    out=raw[:],
                    in_=bass.AP(tensor=biasT.tensor,
                                offset=biasT.offset + kcg * 256 * T + qcc * QCH,
                                ap=[[T, 128], [128 * T, 2], [1, QCH]]))
                raws[kcg] = raw

            thunks = []
            for kcg in range(8):
                def step(kcg=kcg, eb=eb, qcc=qcc, raws=raws):
                    if kcg == 0:
                        dma(0, qcc, raws)
                    if kcg < 7:
                        dma(kcg + 1, qcc, raws)
                    nc.scalar.activation(eb[:, kcg * 2:(kcg + 1) * 2, :],
                                         raws[kcg][:], AF.Exp)
                thunks.append(step)
            return thunks

        # ---------------- pre-phase: k/v projections ----------------
        wkv_pool = tc.alloc_tile_pool(name="wkv", bufs=1)
        xkv_pool = tc.alloc_tile_pool(name="xkv", bufs=4)
        pp = tc.alloc_tile_pool(name="pp", bufs=1, space="PSUM")

        wk_sb = wkv_pool.tile([128, EC, HD], BF16)
        wv_sb = wkv_pool.tile([128, EC, HD], BF16)
        # startup choreography: spread the first loads over all three DMA
        # paths in consumption order so the first kproj/vproj groups never wait
        nc.scalar.dma_start(
            out=wk_sb[:, 0:4, :],
            in_=wk[0:512, :].rearrange("(ec p) n -> p ec n", p=128))
        nc.scalar.dma_start(
            out=wv_sb[:, 0:4, :],
            in_=wv[0:512, :].rearrange("(ec p) n -> p ec n", p=128))
        for eg in (1, 2, 3):
            nc.gpsimd.dma_start(
                out=wk_sb[:, eg * 4:(eg + 1) * 4, :],
                in_=wk[eg * 512:(eg + 1) * 512, :].rearrange("(ec p) n -> p ec n", p=128))

        def load_xkv_quarter(tc4, ecq, eng=None):
            t = xkv_pool.tile([128, 4, HD], BF16, name="xkv_t", tag="xkv")
            (eng or nc.sync).dma_start(
                out=t[:],
                in_=bass.AP(tensor=xkvT.tensor,
                            offset=xkvT.offset + tc4 * 512 + ecq * 4 * 128 * T,
                            ap=[[T, 128], [128 * T, 4], [1, 512]]))
            return t

        pre_bias = (bias_exp_thunks(0) + bias_exp_thunks(1)
                    + bias_exp_thunks(2))
        # depth-2 xkv prefetch alternating over the two hw DMA queues
        pend_q = [load_xkv_quarter(0, 0, nc.sync),
                  load_xkv_quarter(0, 1, nc.scalar)]
        qidx = [2]
        for tc4 in range(4):
            if tc4 == 2:
                for eg in range(4):
                    nc.gpsimd.dma_start(
                        out=wq_sb[:, eg * 4:(eg + 1) * 4, :],
                        in_=wq[eg * 512:(eg + 1) * 512, :].rearrange(
                            "(ec p) n -> p ec n", p=128))
                load_xq(0)
            # 8 concurrently-open psum accumulation groups: kT (4 hdc) + v (4 sub)
            ps_k = [pp.tile([128, 512], F32, name=f"ppk{h}", tag=f"ppk{h}")
                    for h in range(NPAIR)]
            ps_v = [pp.tile([128, 512], F32, name=f"ppv{s}", tag=f"ppv{s}")
                    for s in range(4)]
            for ecq in range(4):
                cur_q = pend_q.pop(0)
                if tc4 == 0 and ecq > 0:
                    nc.scalar.dma_start(
                        out=wv_sb[:, ecq * 4:(ecq + 1) * 4, :],
                        in_=wv[ecq * 512:(ecq + 1) * 512, :].rearrange(
                            "(ec p) n -> p ec n", p=128))
                if qidx[0] < 16:
                    eng = (nc.sync, nc.scalar, nc.gpsimd)[qidx[0] % 3]
                    pend_q.append(load_xkv_quarter(qidx[0] // 4, qidx[0] % 4,
                                                   eng))
                    qidx[0] += 1
                if pre_bias and tc4 >= 1:
                    pre_bias.pop(0)()
                    if tc4 >= 2:
                        pre_bias.pop(0)()
                for hdc in range(NPAIR):
                    for e4 in range(4):
                        ec = ecq * 4 + e4
                        nc.tensor.matmul(ps_k[hdc][:],
                                         lhsT=wk_sb[:, ec, hdc * 128:(hdc + 1) * 128],
                                         rhs=cur_q[:, e4, :],
                                         start=(ec == 0), stop=(ec == EC - 1),
                                         skip_group_check=True)
                for sub in range(4):
                    for e4 in range(4):
                        ec = ecq * 4 + e4
                        nc.tensor.matmul(ps_v[sub][:],
                                         lhsT=cur_q[:, e4, sub * 128:(sub + 1) * 128],
                                         rhs=wv_sb[:, ec, :],
                                         start=(ec == 0), stop=(ec == EC - 1),
                                         skip_group_check=True)
            with tc.high_priority(offset=1_500_000):
                for hdc in range(NPAIR):
                    nc.vector.tensor_copy(
                        kT_sb[:, hdc, tc4 * 512:(tc4 + 1) * 512], ps_k[hdc][:])
                for sub in range(4):
                    nc.vector.tensor_copy(
                        v_sb[:, tc4 * 4 + sub, :, 0:D],
                        ps_v[sub].rearrange("p (h d) -> p h d", h=NH))

        # q-projection for q-chunk 0 (uses the pre-phase psum pool)
        qT_t[0] = qT_pool.tile([128, NPAIR, QCH], BF16, name="qT_t", tag="qT")
        for hdc in range(NPAIR):
            ps = pp.tile([128, 512], F32, name="pp_t", tag=f"ppk{hdc}")
            for ec in range(EC):
                nc.tensor.matmul(ps[:],
                                 lhsT=wq_sb[:, ec, hdc * 128:(hdc + 1) * 128],
                                 rhs=xq_t[0][:, ec, :],
                                 start=(ec == 0), stop=(ec == EC - 1))
            nc.vector.tensor_copy(qT_t[0][:, hdc, :], ps[:])
        for th in pre_bias:
            th()
        load_xq(1)

        pp.release()
        xkv_pool.release()
        wkv_pool.release()

        # ---------------- main attention phase ----------------
        wo_pool = ctx.enter_context(tc.tile_pool(name="wop", bufs=1))
        cpool = tc.alloc_tile_pool(name="cpsum", bufs=3, space="PSUM")
        fpool = tc.alloc_tile_pool(name="fpsum", bufs=1, space="PSUM")
        spool = tc.alloc_tile_pool(name="spsum", bufs=2, space="PSUM")
        atpool = ctx.enter_context(tc.tile_pool(name="atp", bufs=3))
        a2pool = ctx.enter_context(tc.tile_pool(name="a2p", bufs=4))
        ostage_pool = ctx.enter_context(tc.tile_pool(name="osp", bufs=2))
        norm_pool = ctx.enter_context(tc.tile_pool(name="nrm", bufs=1))

        wo_sb = wo_pool.tile([128, NPAIR, E], BF16)

        # ---- fill-work machinery: thunks emitted between attention matmuls ----
        def qproj_thunks(qcc):
            qT_t[qcc] = qT_pool.tile([128, NPAIR, QCH], BF16, name="qT_t", tag="qT")
            thunks = []
            for hdc in range(NPAIR):
                ps_box = [None]
                for ec in range(EC):
                    def mm(hdc=hdc, ec=ec, ps_box=ps_box, qcc=qcc):
                        if ec == 0:
                            ps_box[0] = fpool.tile([128, 512], F32, name="fp_t",
                                                   tag="fill")
                        nc.tensor.matmul(ps_box[0][:],
                                         lhsT=wq_sb[:, ec, hdc * 128:(hdc + 1) * 128],
                                         rhs=xq_t[qcc][:, ec, :],
                                         start=(ec == 0), stop=(ec == EC - 1),
                                         skip_group_check=True)
                    thunks.append(mm)

                def fin(hdc=hdc, ps_box=ps_box, qcc=qcc):
                    nc.scalar.copy(qT_t[qcc][:, hdc, :], ps_box[0][:])
                fin.is_fin = True
                thunks.append(fin)
            return thunks

        def outproj_thunks(qcc):
            """Output projection for q-chunk qcc (reads ctxT_t[qcc])."""
            thunks = []
            for t16 in range(QCH // 128):
                for ncol in range(E // 512):
                    ps_box = [None]
                    for pair in range(NPAIR):
                        def mm(pair=pair, t16=t16, ncol=ncol, ps_box=ps_box, qcc=qcc):
                            if pair == 0:
                                ps_box[0] = fpool.tile([128, 512], F32, name="fp_t",
                                                       tag="fill")
                            nc.tensor.matmul(
                                ps_box[0][:],
                                lhsT=ctxT_t[qcc][:, pair, t16 * 128:(t16 + 1) * 128],
                                rhs=wo_sb[:, pair, ncol * 512:(ncol + 1) * 512],
                                start=(pair == 0), stop=(pair == NPAIR - 1),
                                skip_group_check=True)
                        thunks.append(mm)

                    def fin(t16=t16, ncol=ncol, ps_box=ps_box, qcc=qcc):
                        ot = ostage_pool.tile([128, 512], BF16, name="ot_t", tag="ot")
                        nc.vector.tensor_copy(ot[:], ps_box[0][:])
                        nc.sync.dma_start(
                            out=out[(qcc * 4 + t16) * 128:(qcc * 4 + t16 + 1) * 128,
                                    ncol * 512:(ncol + 1) * 512],
                            in_=ot[:])
                    fin.is_fin = True
                    thunks.append(fin)
            return thunks

        fill_q = []

        def drain_fill(n):
            for _ in range(n):
                if not fill_q:
                    return
                fill_q.pop(0)()

        # ---- flat software pipeline over all (qcc, pair, kc) iterations ----
        flat = [(qcc, pair, kc)
                for qcc in range(NQC)
                for pair in ([1, 2, 3, 0] if qcc == NQC - 1 else range(NPAIR))
                for kc in range(KC)]
        NIT = len(flat)
        a2_ring = [None] * NIT   # a2 tile per iteration, consumed CTX_LAG iters later
        ctx_cur = [None]         # current AU's ctx psum pair

        def emit_scores(i):
            qcc, pair, kc = flat[i]
            with tc.high_priority(offset=1_000_000):
                _emit_scores_prio(i)

        def _emit_scores_prio(i):
            qcc, pair, kc = flat[i]
            s = spool.tile([128, 2 * QCH], F32, name="s_t", tag="s")
            for hh in range(2):
                pr = slice(hh * D, (hh + 1) * D)
                nc.tensor.matmul(
                    s[:, hh * QCH:(hh + 1) * QCH],
                    lhsT=kT_sb[pr, pair, kc * 128:(kc + 1) * 128],
                    rhs=qT_t[qcc][pr, pair, :],
                    start=True, stop=True)
            at = atpool.tile([128, 2 * QCH], BF16, name="at_t", tag="at")
            nc.scalar.activation(at[:], s[:], AF.Exp)
            a2 = a2pool.tile([128, 2 * QCH], BF16, name="a2_t", tag="a2")
            # one merged mul: expb block broadcast (stride-0) over both heads
            ebb = expb_t[qcc][:, kc, :].unsqueeze(1).broadcast_to([128, 2, QCH])
            nc.vector.tensor_mul(
                a2[:].rearrange("p (h q) -> p h q", h=2),
                at[:].rearrange("p (h q) -> p h q", h=2),
                ebb)
            a2_ring[i] = a2

        def emit_ctx(i):
            qcc, pair, kc = flat[i]
            if kc == 0:
                ctx_cur[0] = [cpool.tile([D + 1, QCH], F32, name=f"ctx{hh}",
                                         tag="ctx")
                              for hh in range(2)]
            a2 = a2_ring[i]
            a2_ring[i] = None
            for hh in range(2):
                nc.tensor.matmul(
                    ctx_cur[0][hh][0:D + 1, :],
                    lhsT=v_sb[:, kc, pair * 2 + hh, :],
                    rhs=a2[:, hh * QCH:(hh + 1) * QCH],
                    start=(kc == 0), stop=(kc == KC - 1))
            if kc == KC - 1:
                emit_normalize(qcc, pair, ctx_cur[0])

        def emit_normalize(qcc, pair, ctx_t):
            if (qcc, pair) == (NQC - 1, 0):
                # last AU: its normalize gates the tail -- jump every queue
                with tc.high_priority(offset=2_000_000):
                    _emit_normalize(qcc, pair, ctx_t)
            else:
                _emit_normalize(qcc, pair, ctx_t)

        def _emit_normalize(qcc, pair, ctx_t):
            # normalize ctx rows 0..63 by 1/ctx[64] (the attn row-sums),
            # reading the psum accumulators directly. DVE copies can shift
            # partition base, so the row-sum moves to partition 0 with a tiny
            # copy and the hh1 product writes straight to ctxT rows 64:128.
            # Both recip/broadcast chains are issued before either scale-mul
            # so the second chain's DVE work hides the first broadcast.
            recbs = []
            for hh in range(2):
                s1 = norm_pool.tile([1, QCH], F32, name="sum_t", tag=f"sum{hh}")
                nc.vector.tensor_copy(s1[:], ctx_t[hh][D:D + 1, :])
                rec = norm_pool.tile([1, QCH], F32, name="rec_t", tag=f"rec{hh}")
                nc.vector.reciprocal_approx_fast(out=rec[:], in_=s1[:])
                recb = norm_pool.tile([D, QCH], F32, name="recb_t",
                                      tag=f"recb{hh}")
                nc.gpsimd.partition_broadcast(recb[:], rec[:], channels=D)
                recbs.append(recb)
            for hh in range(2):
                nc.vector.tensor_mul(
                    ctxT_t[qcc][hh * D:(hh + 1) * D, pair, :],
                    ctx_t[hh][0:D, :], recbs[hh][:])

        bias_q = []   # pending bias-exp thunks, drained one per iteration

        prev_qcc = [-1]
        for i in range(NIT + CTX_LAG):
            if i < NIT:
                qcc, pair, kc = flat[i]
                if qcc != prev_qcc[0]:
                    prev_qcc[0] = qcc
                    # new q-chunk: allocate ctxT, queue fill + bias work
                    ctxT_t[qcc] = ctxT_pool.tile([128, NPAIR, QCH], BF16,
                                                 name="ctxT_t", tag="ctxT")
                    if qcc == 0:
                        fill_q.extend(qproj_thunks(1))
                        fill_q.extend(qproj_thunks(2))
                    elif qcc == 1:
                        fill_q.extend(qproj_thunks(3))
                        fill_q.extend(outproj_thunks(0))
                    else:
                        fill_q.extend(outproj_thunks(qcc - 1))
                    if qcc + 2 < NQC:
                        load_xq(qcc + 2)
                if kc == 0 and pair == 2 and qcc == 1:
                    bias_q.extend(bias_exp_thunks(3))
                if qcc == 0 and kc == 0 and pair in (0, 1):
                    nc.gpsimd.dma_start(
                        out=wo_sb[:, 2 * pair:2 * pair + 2, :],
                        in_=wo[pair * 256:(pair + 1) * 256, :].rearrange(
                            "(c p) n -> p c n", p=128))
                emit_scores(i)
                if bias_q and kc % 2 == 1:
                    bias_q.pop(0)()
            if i >= CTX_LAG:
                emit_ctx(i - CTX_LAG)
            # fill rate: drain evenly over the remaining iterations of this
            # q-chunk (+2 lag slack), at least 1, at most 4 per iteration
            rem_it = 16 * NPAIR - (flat[min(i, NIT - 1)][1] * KC +
                                   flat[min(i, NIT - 1)][2])
            rate = max(1, min(4, -(-len(fill_q) // max(1, rem_it))))
            drain_fill(rate)

        drain_fill(10 ** 9)
        spool.release()
        fpool.release()

        # ---------------- tail: output projection for the last q-chunk ----------------
        # tail psum opens in the released scores+fill banks: gated only by the
        # last exp's read, so pair-1/2/3 matmuls overlap the final normalize
        with tc.tile_pool(name="tailp", bufs=5, space="PSUM") as tailp:
            for gi in range(16):
                t16, ncol = gi // 4, gi % 4
                po = tailp.tile([128, 512], F32, name="po_t", tag="po")
                for j, pair in enumerate([1, 2, 3, 0]):
                    nc.tensor.matmul(
                        po[:],
                        lhsT=ctxT_t[NQC - 1][:, pair, t16 * 128:(t16 + 1) * 128],
                        rhs=wo_sb[:, pair, ncol * 512:(ncol + 1) * 512],
                        start=(j == 0), stop=(j == NPAIR - 1),
                        skip_group_check=True)
                ot = ostage_pool.tile([128, 512], BF16, name="ot_t", tag="ot")
                if gi % 2 == 0:
                    nc.vector.tensor_copy(ot[:], po[:])
                else:
                    nc.scalar.copy(ot[:], po[:])
                (nc.scalar if gi % 2 == 0 else nc.sync).dma_start(
                    out=out[((NQC - 1) * 4 + t16) * 128:
                            ((NQC - 1) * 4 + t16 + 1) * 128,
                            ncol * 512:(ncol + 1) * 512],
                    in_=ot[:])

        cpool.release()

    nc.compile()
    return nc


_NC_CACHE = {}


def kernel(inputs_q, inputs_kv, bias, wq, wk, wv, wo):
    bf16 = ml_dtypes.bfloat16
    inputs_q = np.asarray(inputs_q)
    inputs_kv = np.asarray(inputs_kv)
    bias = np.asarray(bias)
    # fold the reference's 1/sqrt(D) query scaling into wq
    wq_s = (np.asarray(wq).reshape(E, H * D) / np.sqrt(D)).astype(bf16)
    wk_s = np.asarray(wk).reshape(E, H * D).astype(bf16)
    wv_s = np.asarray(wv).reshape(E, H * D).astype(bf16)
    wo_s = np.asarray(wo).reshape(H * D, E).astype(bf16)

    # host-side layout marshaling: the kernel wants embed-major activations
    # and key-major bias (pure transposes, no math)
    xq_b = [np.ascontiguousarray(inputs_q[b].T).astype(bf16) for b in range(B)]
    xkv_b = [np.ascontiguousarray(inputs_kv[b].T).astype(bf16) for b in range(B)]
    bias_b = [np.ascontiguousarray(bias[b, 0].T).astype(bf16) for b in range(B)]

    in_maps = []
    for c in range(N_CORES):
        b, hg = c // 4, c % 4
        hs = slice(hg * HD, (hg + 1) * HD)
        in_maps.append({
            "xqT": xq_b[b],
            "xkvT": xkv_b[b],
            "biasT": bias_b[b],
            "wq": np.ascontiguousarray(wq_s[:, hs]),
            "wk": np.ascontiguousarray(wk_s[:, hs]),
            "wv": np.ascontiguousarray(wv_s[:, hs]),
            "wo": np.ascontiguousarray(wo_s[hs, :]),
        })

    if "nc" not in _NC_CACHE:
        _NC_CACHE["nc"] = build_program()
    nc = _NC_CACHE["nc"]

    res = run_bass_kernel_spmd(nc, in_maps, list(range(N_CORES)))
    outs = [np.asarray(r["out"], dtype=np.float32) for r in res.results]
    full = np.empty((B, T, E), dtype=np.float32)
    for b in range(B):
        full[b] = outs[4 * b] + outs[4 * b + 1] + outs[4 * b + 2] + outs[4 * b + 3]
    return full



# revision 50
# speedup vs baseline: 1.2018x; 1.0183x over previous
"""Multi-head dot-product attention (B=2, Q=K=2048, EMB=2048, H=32, D=64) on 8 TRN2 cores.

Sharding: data parallel over batch (2) x tensor parallel over heads (4 groups of 8).
Core c handles batch c//4, heads 8*(c%4) .. 8*(c%4)+8. Each core computes a partial
output [2048, 2048] (its heads' contribution through wo); host sums the 4 head-group
partials per batch.

Single software-pipelined program per core (all matmuls bf16, T=2048 tokens,
HD=512 = 8 heads x 64):
  pre-phase: k^T/v projections (PE-dense; xkv streamed in quarter-chunks through
             8 concurrently-open psum accumulation groups) with exp(bias^T) on
             ScalarE underneath, plus q^T projection for the first q-chunk.
  main:      one flat pipeline over 256 iterations (4 q-chunks x 4 head-pairs x
             16 key-chunks). Per iteration: scores s^T[k,q] for the head pair
             (row-packed K=64 matmul pair), one merged exp on ScalarE, exp(bias)
             mul on DVE; the ctx^T accumulation (ones-augmented v, M=65) trails
             two iterations behind so ScalarE never waits on the PE queue.
             PE idle slots are filled with the remaining q-projection and the
             output-projection matmul groups, a few matmuls at a time.
  tail:      output projection for the last q-chunk.
Output partials are written bf16; the host sums them in fp32.
"""

import numpy as np
import ml_dtypes
from contextlib import ExitStack

import concourse.bass as bass
from concourse import bacc
import concourse.mybir as mybir
import concourse.tile as tile
from concourse.bass_utils import run_bass_kernel_spmd

BF16 = mybir.dt.bfloat16
F32 = mybir.dt.float32
AF = mybir.ActivationFunctionType

B, T, E = 2, 2048, 2048          # batch, tokens (Q=K), embed
H, D = 32, 64                     # total heads, head dim
NH = 8                            # heads per core
HD = NH * D                       # 512, per-core head-dim total
EC = E // 128                     # 16 contraction chunks
KC = T // 128                     # 16 key chunks
QCH = 512                         # attention q-chunk
NQC = T // QCH                    # 4 q-chunks
NPAIR = NH // 2                   # 4 head pairs
N_CORES = 8


def build_program():
    nc = bacc.Bacc("TRN2", target_bir_lowering=False, debug=False,
                   num_devices=N_CORES)

    xqT = nc.dram_tensor("xqT", [E, T], BF16, kind="ExternalInput").ap()
    xkvT = nc.dram_tensor("xkvT", [E, T], BF16, kind="ExternalInput").ap()
    biasT = nc.dram_tensor("biasT", [T, T], BF16, kind="ExternalInput").ap()
    wq = nc.dram_tensor("wq", [E, HD], BF16, kind="ExternalInput").ap()
    wk = nc.dram_tensor("wk", [E, HD], BF16, kind="ExternalInput").ap()
    wv = nc.dram_tensor("wv", [E, HD], BF16, kind="ExternalInput").ap()
    wo = nc.dram_tensor("wo", [HD, E], BF16, kind="ExternalInput").ap()
    out = nc.dram_tensor("out", [T, E], BF16, kind="ExternalOutput").ap()

    with tile.TileContext(nc) as tc, ExitStack() as ctx:
        persist = ctx.enter_context(tc.tile_pool(name="persist", bufs=1))
        kT_sb = persist.tile([128, NPAIR, T], BF16)       # k^T[d(2 heads), pair, t]
        v_sb = persist.tile([128, KC, NH, D + 1], BF16)   # v[k, kc, h, d] + ones col
        nc.vector.memset(v_sb[:, :, :, D:D + 1], 1.0)

        qT_pool = ctx.enter_context(tc.tile_pool(name="qtp", bufs=3))
        ctxT_pool = ctx.enter_context(tc.tile_pool(name="ctp", bufs=2))
        expb_pool = ctx.enter_context(tc.tile_pool(name="ebp", bufs=3))
        CTX_LAG = 2               # iterations between scores and ctx consumption
        rawb_pool = ctx.enter_context(tc.tile_pool(name="rbp", bufs=3))
        wq_pool = ctx.enter_context(tc.tile_pool(name="wqp", bufs=1))
        xq_pool = ctx.enter_context(tc.tile_pool(name="xqp", bufs=2))

        qT_t = [None] * NQC       # [128, NPAIR, QCH] bf16 per q-chunk
        ctxT_t = [None] * NQC     # [128, NPAIR, QCH] bf16 per q-chunk
        expb_t = [None] * NQC     # [128, KC, QCH] bf16 per q-chunk
        xq_t = [None] * NQC

        wq_sb = wq_pool.tile([128, EC, HD], BF16)

        def load_xq(qcc):
            t = xq_pool.tile([128, EC, QCH], BF16, name="xq_t", tag="xq")
            for half in range(2):
                nc.sync.dma_start(
                    out=t[:, half * 8:(half + 1) * 8, :],
                    in_=bass.AP(tensor=xqT.tensor,
                                offset=xqT.offset + qcc * QCH + half * 8 * 128 * T,
                                ap=[[T, 128], [128 * T, 8], [1, QCH]]))
            xq_t[qcc] = t

        def bias_exp_thunks(qcc):
            """Thunks: DMA raw bias columns for q-chunk qcc + exp into expb.
            The DMA for step k issues one thunk ahead of its exp so the exp
            never waits on its DMA inside ScalarE's in-order queue."""
            eb = expb_pool.tile([128, KC, QCH], BF16, name="expb_t", tag="expb")
            expb_t[qcc] = eb
            raws = [None] * 8

            def dma(kcg, qcc=qcc, raws=raws):
                raw = rawb_pool.tile([128, 2, QCH], BF16, name="rawb", tag="raw")
                nc.gpsimd.dma_start(
                    out=raw[:],
                    in_=bass.AP(tensor=biasT.tensor,
                                offset=biasT.offset + kcg * 256 * T + qcc * QCH,
                                ap=[[T, 128], [128 * T, 2], [1, QCH]]))
                raws[kcg] = raw

            thunks = []
            for kcg in range(8):
                def step(kcg=kcg, eb=eb, qcc=qcc, raws=raws):
                    if kcg == 0:
                        dma(0, qcc, raws)
                    if kcg < 7:
                        dma(kcg + 1, qcc, raws)
                    nc.scalar.activation(eb[:, kcg * 2:(kcg + 1) * 2, :],
                                         raws[kcg][:], AF.Exp)
                thunks.append(step)
            return thunks

        # ---------------- pre-phase: k/v projections ----------------
        wkv_pool = tc.alloc_tile_pool(name="wkv", bufs=1)
        xkv_pool = tc.alloc_tile_pool(name="xkv", bufs=4)
        pp = tc.alloc_tile_pool(name="pp", bufs=1, space="PSUM")

        wk_sb = wkv_pool.tile([128, EC, HD], BF16)
        wv_sb = wkv_pool.tile([128, EC, HD], BF16)
        # startup choreography: spread the first loads over all three DMA
        # paths in consumption order so the first kproj/vproj groups never wait
        nc.scalar.dma_start(
            out=wk_sb[:, 0:4, :],
            in_=wk[0:512, :].rearrange("(ec p) n -> p ec n", p=128))
        nc.scalar.dma_start(
            out=wv_sb[:, 0:4, :],
            in_=wv[0:512, :].rearrange("(ec p) n -> p ec n", p=128))
        for eg in (1, 2, 3):
            nc.gpsimd.dma_start(
                out=wk_sb[:, eg * 4:(eg + 1) * 4, :],
                in_=wk[eg * 512:(eg + 1) * 512, :].rearrange("(ec p) n -> p ec n", p=128))

        def load_xkv_quarter(tc4, ecq, eng=None):
            t = xkv_pool.tile([128, 4, HD], BF16, name="xkv_t", tag="xkv")
            (eng or nc.sync).dma_start(
                out=t[:],
                in_=bass.AP(tensor=xkvT.tensor,
                            offset=xkvT.offset + tc4 * 512 + ecq * 4 * 128 * T,
                            ap=[[T, 128], [128 * T, 4], [1, 512]]))
            return t

        pre_bias = (bias_exp_thunks(0) + bias_exp_thunks(1)
                    + bias_exp_thunks(2))
        # depth-2 xkv prefetch alternating over the two hw DMA queues
        pend_q = [load_xkv_quarter(0, 0, nc.sync),
                  load_xkv_quarter(0, 1, nc.scalar)]
        qidx = [2]
        for tc4 in range(4):
            if tc4 == 2:
                for eg in range(4):
                    nc.gpsimd.dma_start(
                        out=wq_sb[:, eg * 4:(eg + 1) * 4, :],
                        in_=wq[eg * 512:(eg + 1) * 512, :].rearrange(
                            "(ec p) n -> p ec n", p=128))
                load_xq(0)
            # 8 concurrently-open psum accumulation groups: kT (4 hdc) + v (4 sub)
            ps_k = [pp.tile([128, 512], F32, name=f"ppk{h}", tag=f"ppk{h}")
                    for h in range(NPAIR)]
            ps_v = [pp.tile([128, 512], F32, name=f"ppv{s}", tag=f"ppv{s}")
                    for s in range(4)]
            for ecq in range(4):
                cur_q = pend_q.pop(0)
                if tc4 == 0 and ecq > 0:
                    nc.scalar.dma_start(
                        out=wv_sb[:, ecq * 4:(ecq + 1) * 4, :],
                        in_=wv[ecq * 512:(ecq + 1) * 512, :].rearrange(
                            "(ec p) n -> p ec n", p=128))
                if qidx[0] < 16:
                    eng = (nc.sync, nc.scalar, nc.gpsimd)[qidx[0] % 3]
                    pend_q.append(load_xkv_quarter(qidx[0] // 4, qidx[0] % 4,
                                                   eng))
                    qidx[0] += 1
                if pre_bias and tc4 >= 1:
                    pre_bias.pop(0)()
                    if tc4 >= 2:
                        pre_bias.pop(0)()
                for hdc in range(NPAIR):
                    for e4 in range(4):
                        ec = ecq * 4 + e4
                        nc.tensor.matmul(ps_k[hdc][:],
                                         lhsT=wk_sb[:, ec, hdc * 128:(hdc + 1) * 128],
                                         rhs=cur_q[:, e4, :],
                                         start=(ec == 0), stop=(ec == EC - 1),
                                         skip_group_check=True)
                for sub in range(4):
                    for e4 in range(4):
                        ec = ecq * 4 + e4
                        nc.tensor.matmul(ps_v[sub][:],
                                         lhsT=cur_q[:, e4, sub * 128:(sub + 1) * 128],
                                         rhs=wv_sb[:, ec, :],
                                         start=(ec == 0), stop=(ec == EC - 1),
                                         skip_group_check=True)
            with tc.high_priority(offset=1_500_000):
                for hdc in range(NPAIR):
                    nc.vector.tensor_copy(
                        kT_sb[:, hdc, tc4 * 512:(tc4 + 1) * 512], ps_k[hdc][:])
                for sub in range(4):
                    nc.vector.tensor_copy(
                        v_sb[:, tc4 * 4 + sub, :, 0:D],
                        ps_v[sub].rearrange("p (h d) -> p h d", h=NH))

        # q-projection for q-chunk 0 (uses the pre-phase psum pool)
        qT_t[0] = qT_pool.tile([128, NPAIR, QCH], BF16, name="qT_t", tag="qT")
        for hdc in range(NPAIR):
            ps = pp.tile([128, 512], F32, name="pp_t", tag=f"ppk{hdc}")
            for ec in range(EC):
                nc.tensor.matmul(ps[:],
                                 lhsT=wq_sb[:, ec, hdc * 128:(hdc + 1) * 128],
                                 rhs=xq_t[0][:, ec, :],
                                 start=(ec == 0), stop=(ec == EC - 1))
            nc.vector.tensor_copy(qT_t[0][:, hdc, :], ps[:])
        for th in pre_bias:
            th()
        load_xq(1)

        pp.release()
        xkv_pool.release()
        wkv_pool.release()

        # ---------------- main attention phase ----------------
        wo_pool = ctx.enter_context(tc.tile_pool(name="wop", bufs=1))
        cpool = tc.alloc_tile_pool(name="cpsum", bufs=3, space="PSUM")
        fpool = tc.alloc_tile_pool(name="fpsum", bufs=1, space="PSUM")
        spool = tc.alloc_tile_pool(name="spsum", bufs=2, space="PSUM")
        atpool = ctx.enter_context(tc.tile_pool(name="atp", bufs=4))
        a2pool = ctx.enter_context(tc.tile_pool(name="a2p", bufs=5))
        ostage_pool = ctx.enter_context(tc.tile_pool(name="osp", bufs=2))
        norm_pool = ctx.enter_context(tc.tile_pool(name="nrm", bufs=1))

        wo_sb = wo_pool.tile([128, NPAIR, E], BF16)

        # ---- fill-work machinery: thunks emitted between attention matmuls ----
        def qproj_thunks(qcc):
            qT_t[qcc] = qT_pool.tile([128, NPAIR, QCH], BF16, name="qT_t", tag="qT")
            thunks = []
            for hdc in range(NPAIR):
                ps_box = [None]
                for ec in range(EC):
                    def mm(hdc=hdc, ec=ec, ps_box=ps_box, qcc=qcc):
                        if ec == 0:
                            ps_box[0] = fpool.tile([128, 512], F32, name="fp_t",
                                                   tag="fill")
                        nc.tensor.matmul(ps_box[0][:],
                                         lhsT=wq_sb[:, ec, hdc * 128:(hdc + 1) * 128],
                                         rhs=xq_t[qcc][:, ec, :],
                                         start=(ec == 0), stop=(ec == EC - 1),
                                         skip_group_check=True)
                    thunks.append(mm)

                def fin(hdc=hdc, ps_box=ps_box, qcc=qcc):
                    nc.scalar.copy(qT_t[qcc][:, hdc, :], ps_box[0][:])
                fin.is_fin = True
                thunks.append(fin)
            return thunks

        def outproj_thunks(qcc):
            """Output projection for q-chunk qcc (reads ctxT_t[qcc])."""
            thunks = []
            for t16 in range(QCH // 128):
                for ncol in range(E // 512):
                    ps_box = [None]
                    for pair in range(NPAIR):
                        def mm(pair=pair, t16=t16, ncol=ncol, ps_box=ps_box, qcc=qcc):
                            if pair == 0:
                                ps_box[0] = fpool.tile([128, 512], F32, name="fp_t",
                                                       tag="fill")
                            nc.tensor.matmul(
                                ps_box[0][:],
                                lhsT=ctxT_t[qcc][:, pair, t16 * 128:(t16 + 1) * 128],
                                rhs=wo_sb[:, pair, ncol * 512:(ncol + 1) * 512],
                                start=(pair == 0), stop=(pair == NPAIR - 1),
                                skip_group_check=True)
                        thunks.append(mm)

                    def fin(t16=t16, ncol=ncol, ps_box=ps_box, qcc=qcc):
                        ot = ostage_pool.tile([128, 512], BF16, name="ot_t", tag="ot")
                        nc.vector.tensor_copy(ot[:], ps_box[0][:])
                        nc.sync.dma_start(
                            out=out[(qcc * 4 + t16) * 128:(qcc * 4 + t16 + 1) * 128,
                                    ncol * 512:(ncol + 1) * 512],
                            in_=ot[:])
                    fin.is_fin = True
                    thunks.append(fin)
            return thunks

        fill_q = []

        def drain_fill(n):
            for _ in range(n):
                if not fill_q:
                    return
                fill_q.pop(0)()

        # ---- flat software pipeline over all (qcc, pair, kc) iterations ----
        flat = [(qcc, pair, kc)
                for qcc in range(NQC)
                for pair in ([1, 2, 3, 0] if qcc == NQC - 1 else range(NPAIR))
                for kc in range(KC)]
        NIT = len(flat)
        a2_ring = [None] * NIT   # a2 tile per iteration, consumed CTX_LAG iters later
        ctx_cur = [None]         # current AU's ctx psum pair

        def emit_scores(i):
            qcc, pair, kc = flat[i]
            with tc.high_priority(offset=1_000_000):
                _emit_scores_prio(i)

        def _emit_scores_prio(i):
            qcc, pair, kc = flat[i]
            s = spool.tile([128, 2 * QCH], F32, name="s_t", tag="s")
            for hh in range(2):
                pr = slice(hh * D, (hh + 1) * D)
                nc.tensor.matmul(
                    s[:, hh * QCH:(hh + 1) * QCH],
                    lhsT=kT_sb[pr, pair, kc * 128:(kc + 1) * 128],
                    rhs=qT_t[qcc][pr, pair, :],
                    start=True, stop=True)
            at = atpool.tile([128, 2 * QCH], BF16, name="at_t", tag="at")
            nc.scalar.activation(at[:], s[:], AF.Exp)
            a2 = a2pool.tile([128, 2 * QCH], BF16, name="a2_t", tag="a2")
            # one merged mul: expb block broadcast (stride-0) over both heads
            ebb = expb_t[qcc][:, kc, :].unsqueeze(1).broadcast_to([128, 2, QCH])
            nc.vector.tensor_mul(
                a2[:].rearrange("p (h q) -> p h q", h=2),
                at[:].rearrange("p (h q) -> p h q", h=2),
                ebb)
            a2_ring[i] = a2

        def emit_ctx(i):
            qcc, pair, kc = flat[i]
            if kc == 0:
                ctx_cur[0] = [cpool.tile([D + 1, QCH], F32, name=f"ctx{hh}",
                                         tag="ctx")
                              for hh in range(2)]
            a2 = a2_ring[i]
            a2_ring[i] = None
            for hh in range(2):
                nc.tensor.matmul(
                    ctx_cur[0][hh][0:D + 1, :],
                    lhsT=v_sb[:, kc, pair * 2 + hh, :],
                    rhs=a2[:, hh * QCH:(hh + 1) * QCH],
                    start=(kc == 0), stop=(kc == KC - 1))
            if kc == KC - 1:
                emit_normalize(qcc, pair, ctx_cur[0])

        def emit_normalize(qcc, pair, ctx_t):
            if (qcc, pair) == (NQC - 1, 0):
                # last AU: its normalize gates the tail -- jump every queue
                with tc.high_priority(offset=2_000_000):
                    _emit_normalize(qcc, pair, ctx_t)
            else:
                _emit_normalize(qcc, pair, ctx_t)

        def _emit_normalize(qcc, pair, ctx_t):
            # normalize ctx rows 0..63 by 1/ctx[64] (the attn row-sums),
            # reading the psum accumulators directly. DVE copies can shift
            # partition base, so the row-sum moves to partition 0 with a tiny
            # copy and the hh1 product writes straight to ctxT rows 64:128.
            # Both recip/broadcast chains are issued before either scale-mul
            # so the second chain's DVE work hides the first broadcast.
            recbs = []
            for hh in range(2):
                s1 = norm_pool.tile([1, QCH], F32, name="sum_t", tag=f"sum{hh}")
                nc.vector.tensor_copy(s1[:], ctx_t[hh][D:D + 1, :])
                rec = norm_pool.tile([1, QCH], F32, name="rec_t", tag=f"rec{hh}")
                nc.vector.reciprocal_approx_fast(out=rec[:], in_=s1[:])
                recb = norm_pool.tile([D, QCH], F32, name="recb_t",
                                      tag=f"recb{hh}")
                nc.gpsimd.partition_broadcast(recb[:], rec[:], channels=D)
                recbs.append(recb)
            for hh in range(2):
                nc.vector.tensor_mul(
                    ctxT_t[qcc][hh * D:(hh + 1) * D, pair, :],
                    ctx_t[hh][0:D, :], recbs[hh][:])

        bias_q = []   # pending bias-exp thunks, drained one per iteration

        prev_qcc = [-1]
        for i in range(NIT + CTX_LAG):
            if i < NIT:
                qcc, pair, kc = flat[i]
                if qcc != prev_qcc[0]:
                    prev_qcc[0] = qcc
                    # new q-chunk: allocate ctxT, queue fill + bias work
                    ctxT_t[qcc] = ctxT_pool.tile([128, NPAIR, QCH], BF16,
                                                 name="ctxT_t", tag="ctxT")
                    if qcc == 0:
                        fill_q.extend(qproj_thunks(1))
                        fill_q.extend(qproj_thunks(2))
                    elif qcc == 1:
                        fill_q.extend(qproj_thunks(3))
                        fill_q.extend(outproj_thunks(0))
                    else:
                        fill_q.extend(outproj_thunks(qcc - 1))
                    if qcc + 2 < NQC:
                        load_xq(qcc + 2)
                if kc == 0 and pair == 2 and qcc == 1:
                    bias_q.extend(bias_exp_thunks(3))
                if qcc == 0 and kc == 0 and pair in (0, 1):
                    nc.gpsimd.dma_start(
                        out=wo_sb[:, 2 * pair:2 * pair + 2, :],
                        in_=wo[pair * 256:(pair + 1) * 256, :].rearrange(
                            "(c p) n -> p c n", p=128))
                emit_scores(i)
                if bias_q and kc % 2 == 1:
                    bias_q.pop(0)()
            if i >= CTX_LAG:
                emit_ctx(i - CTX_LAG)
            # fill rate: drain evenly over the remaining iterations of this
            # q-chunk (+2 lag slack), at least 1, at most 4 per iteration
            rem_it = 16 * NPAIR - (flat[min(i, NIT - 1)][1] * KC +
                                   flat[min(i, NIT - 1)][2])
            rate = max(1, min(4, -(-len(fill_q) // max(1, rem_it))))
            drain_fill(rate)

        drain_fill(10 ** 9)
        spool.release()
        fpool.release()

        # ---------------- tail: output projection for the last q-chunk ----------------
        # tail psum opens in the released scores+fill banks: gated only by the
        # last exp's read, so pair-1/2/3 matmuls overlap the final normalize
        with tc.tile_pool(name="tailp", bufs=5, space="PSUM") as tailp:
            for gi in range(16):
                t16, ncol = gi // 4, gi % 4
                po = tailp.tile([128, 512], F32, name="po_t", tag="po")
                for j, pair in enumerate([1, 2, 3, 0]):
                    nc.tensor.matmul(
                        po[:],
                        lhsT=ctxT_t[NQC - 1][:, pair, t16 * 128:(t16 + 1) * 128],
                        rhs=wo_sb[:, pair, ncol * 512:(ncol + 1) * 512],
                        start=(j == 0), stop=(j == NPAIR - 1),
                        skip_group_check=True)
                ot = ostage_pool.tile([128, 512], BF16, name="ot_t", tag="ot")
                if gi % 2 == 0:
                    nc.vector.tensor_copy(ot[:], po[:])
                else:
                    nc.scalar.copy(ot[:], po[:])
                (nc.scalar if gi % 2 == 0 else nc.sync).dma_start(
                    out=out[((NQC - 1) * 4 + t16) * 128:
                            ((NQC - 1) * 4 + t16 + 1) * 128,
                            ncol * 512:(ncol + 1) * 512],
                    in_=ot[:])

        cpool.release()

    nc.compile()
    return nc


_NC_CACHE = {}


def kernel(inputs_q, inputs_kv, bias, wq, wk, wv, wo):
    bf16 = ml_dtypes.bfloat16
    inputs_q = np.asarray(inputs_q)
    inputs_kv = np.asarray(inputs_kv)
    bias = np.asarray(bias)
    # fold the reference's 1/sqrt(D) query scaling into wq
    wq_s = (np.asarray(wq).reshape(E, H * D) / np.sqrt(D)).astype(bf16)
    wk_s = np.asarray(wk).reshape(E, H * D).astype(bf16)
    wv_s = np.asarray(wv).reshape(E, H * D).astype(bf16)
    wo_s = np.asarray(wo).reshape(H * D, E).astype(bf16)

    # host-side layout marshaling: the kernel wants embed-major activations
    # and key-major bias (pure transposes, no math)
    xq_b = [np.ascontiguousarray(inputs_q[b].T).astype(bf16) for b in range(B)]
    xkv_b = [np.ascontiguousarray(inputs_kv[b].T).astype(bf16) for b in range(B)]
    bias_b = [np.ascontiguousarray(bias[b, 0].T).astype(bf16) for b in range(B)]

    in_maps = []
    for c in range(N_CORES):
        b, hg = c // 4, c % 4
        hs = slice(hg * HD, (hg + 1) * HD)
        in_maps.append({
            "xqT": xq_b[b],
            "xkvT": xkv_b[b],
            "biasT": bias_b[b],
            "wq": np.ascontiguousarray(wq_s[:, hs]),
            "wk": np.ascontiguousarray(wk_s[:, hs]),
            "wv": np.ascontiguousarray(wv_s[:, hs]),
            "wo": np.ascontiguousarray(wo_s[hs, :]),
        })

    if "nc" not in _NC_CACHE:
        _NC_CACHE["nc"] = build_program()
    nc = _NC_CACHE["nc"]

    res = run_bass_kernel_spmd(nc, in_maps, list(range(N_CORES)))
    outs = [np.asarray(r["out"], dtype=np.float32) for r in res.results]
    full = np.empty((B, T, E), dtype=np.float32)
    for b in range(B):
        full[b] = outs[4 * b] + outs[4 * b + 1] + outs[4 * b + 2] + outs[4 * b + 3]
    return full

